# revision 13
# baseline (speedup 1.0000x reference)
"""Trainium2 Bass kernel for the MLPConstructor2 adjacency problem.

Computes, per batch b (one NeuronCore each, 8-way data parallel over B):
    adj[i, j] = tanh(relu(x1_i @ w1 + x2_j @ w2 + b))
for the four (spatial/temporal) quadrants of a (2560, 2560) output,
stored as bf16 (widened to f32 on the host; quantization error ~2e-3
against a 2e-2 gate).

The output is an outer broadcast-sum of per-row and per-column scalar
vectors. With a bf16 store the kernel is ScalarE-bound: every output
element takes one tanh slot (1 elem/cycle/lane at 1.2 GHz -> ~43 us
floor), with the 13.1 MB/core store (~40 us at ~330 GB/s) just under
it. Design:

- x is staged twice, in (t p) layout for the row-side stats (so each
  128-row output tile's biases land on partitions directly) and in (p t)
  layout for the col-side stats (so the stat tile streams out to a DRAM
  scratch contiguously in row order -- no transpose anywhere).
- Dot-product stats are mul/reduce on VectorE; quadrant biases are
  folded into the row-side stats (off the col critical path).
- The column vectors are replicated across partitions with a single
  partition-step-0 DMA broadcast-load of the scratch: pure DMA.
- Ring discipline: the Scalar sequencer issues nothing before its
  activations (a dummy [128,1] tanh preloads the act table at t=0); the
  Sync ring carries only the critical chain (x_sp stages, scratch
  stores, col broadcast-loads) then even-tile stores; weight/bias
  broadcasts go to the idle Tensor ring; x_tm stages and odd-tile
  stores to the GpSimd (SWDGE) ring; the last tile's store is split
  four ways across rings to shrink the drain tail.
- Main loop per 128-row output tile: 2 ScalarE tanh activations
  (per-quadrant per-partition row bias), 1 VectorE bf16 relu in place,
  one 655 KB contiguous bf16 store.
"""

import numpy as np
from contextlib import ExitStack

import concourse.bacc as bacc
import concourse.mybir as mybir
import concourse.tile as tile
from concourse.bass_utils import run_bass_kernel_spmd

B, N, T, D = 8, 2048, 512, 32
W = N + T            # 2560
NT, TT = N // 128, T // 128   # 16, 4 row-tiles
F32 = mybir.dt.float32
BF16 = mybir.dt.bfloat16
QUADS = ("ss", "st", "ts", "tt")


def _emit(tc, sp, tm, ws, scr, adj):
    nc = tc.nc
    AF = mybir.ActivationFunctionType
    OP = mybir.AluOpType
    with ExitStack() as ctx:
        const = ctx.enter_context(tc.tile_pool(name="const", bufs=1))
        outp = ctx.enter_context(tc.tile_pool(name="outp", bufs=8))

        # dummy tanh: pulls ACT_TABLE_LOAD off the first real TANH
        dummy = const.tile([128, 1], F32, name="dummy")
        nc.vector.memset(dummy[:], 0.0)
        nc.scalar.activation(dummy[:], dummy[:], AF.Tanh)

        # ---- stage inputs -------------------------------------------------
        # (p t): row p*nt+t at [p, t*D:(t+1)*D]; (t p): row t*128+p there.
        # All four on the Sync ring ahead of the scratch/broadcast chain.
        x_sp_pt = const.tile([128, NT * D], F32, name="x_sp_pt")
        nc.sync.dma_start(x_sp_pt[:], sp.rearrange("(p t) d -> p t d", p=128))
        x_sp_tp = const.tile([128, NT * D], F32, name="x_sp_tp")
        nc.sync.dma_start(x_sp_tp[:], sp.rearrange("(t p) d -> p t d", p=128))
        x_tm_pt = const.tile([128, TT * D], F32, name="x_tm_pt")
        nc.sync.dma_start(x_tm_pt[:], tm.rearrange("(p t) d -> p t d", p=128))
        x_tm_tp = const.tile([128, TT * D], F32, name="x_tm_tp")
        nc.sync.dma_start(x_tm_tp[:], tm.rearrange("(t p) d -> p t d", p=128))

        # broadcast weights straight from DRAM with step-0 partition APs.
        # HWDGE broadcasts are ~0.7us each vs ~3.2us on SWDGE (128 sw
        # descriptors), so the early ones ride the Scalar ring, which is
        # otherwise idle until its first TANH at ~16us; the tm-side pair
        # (needed ~30us in) takes the slow GpSimd ring.
        # col-side pairs: wc_sp = [w_ss2, w_ts2], wc_tm = [w_st2, w_tt2]
        # row-side pairs: wr_sp = [w_ss1, w_st1], wr_tm = [w_ts1, w_tt1]
        def wload(name, spec, eng):
            t = const.tile([128, 2 * D], F32, name=name, tag=name)
            for i, (nm, half) in enumerate(spec):
                src = ws[f"w_{nm}"][half * D : (half + 1) * D]
                eng.dma_start(
                    t[:, i * D : (i + 1) * D], src.unsqueeze(0).broadcast_to((128, D))
                )
            return t

        wc_sp = wload("wc_sp", [("ss", 1), ("ts", 1)], nc.scalar)
        wr_sp = wload("wr_sp", [("ss", 0), ("st", 0)], nc.scalar)
        bb = const.tile([128, 4], F32, name="bb")   # b_ss, b_st, b_ts, b_tt
        for j, nm in enumerate(QUADS):
            nc.scalar.dma_start(
                bb[:, j : j + 1], ws[f"b_{nm}"].unsqueeze(0).broadcast_to((128, 1))
            )
        wc_tm = wload("wc_tm", [("st", 1), ("tt", 1)], nc.gpsimd)
        wr_tm = wload("wr_tm", [("ts", 0), ("tt", 0)], nc.gpsimd)

        # ---- stats on VectorE: mul + reduce over D ------------------------
        # col tiles and the scratch bounce are bf16: halves the broadcast
        # transfer on the critical path (~2e-3 extra quantization pre-tanh).
        col_sp = const.tile([128, W], BF16, name="col_sp")
        col_tm = const.tile([128, W], BF16, name="col_tm")

        def cstat(x, nt, w, scr_rng, col_dst, name):
            # col-side slot in (p t) layout: mul/reduce, bounce through DRAM
            # scratch, partition-broadcast reload into col_dst.
            prod = const.tile([128, nt * D], F32, name=f"cprod_{name}")
            x3 = x[:].rearrange("p (t d) -> p t d", t=nt)
            p3 = prod[:].rearrange("p (t d) -> p t d", t=nt)
            nc.vector.tensor_tensor(
                p3, x3, w.unsqueeze(1).broadcast_to((128, nt, D)), OP.mult
            )
            st = const.tile([128, nt], BF16, name=f"cstat_{name}")
            with nc.allow_low_precision(reason="bf16 col stats; D=32 sum, ~2e-3 err vs 2e-2 gate"):
                nc.vector.tensor_reduce(st[:], p3, axis=mybir.AxisListType.X, op=OP.add)
            nc.sync.dma_start(scr_rng, st[:])
            nc.sync.dma_start(
                col_dst, scr_rng.unsqueeze(0).broadcast_to((128, scr_rng.shape[0]))
            )

        def rstat_slot(x, nt, w, bias, dst, name):
            # row-side slot in (t p) layout, quadrant bias folded in
            prod = const.tile([128, nt * D], F32, name=f"rprod_{name}")
            x3 = x[:].rearrange("p (t d) -> p t d", t=nt)
            p3 = prod[:].rearrange("p (t d) -> p t d", t=nt)
            nc.vector.tensor_tensor(
                p3, x3, w.unsqueeze(1).broadcast_to((128, nt, D)), OP.mult
            )
            nc.vector.tensor_reduce(dst, p3, axis=mybir.AxisListType.X, op=OP.add)
            nc.vector.tensor_scalar_add(dst, dst, bias)

        r_sp = const.tile([128, 2 * NT], F32, name="r_sp")
        r_tm = const.tile([128, 2 * TT], F32, name="r_tm")

        # interleave so the first-TANH dependencies (col_sp[0:N], r_sp slot
        # ss) complete first, and each col slot's store/broadcast fires as
        # soon as its stat lands.
        cstat(x_sp_pt, NT, wc_sp[:, 0:D], scr["sp"][0:N], col_sp[:, 0:N], "ss")
        rstat_slot(x_sp_tp, NT, wr_sp[:, 0:D], bb[:, 0:1], r_sp[:, 0:NT], "ss")
        cstat(x_tm_pt, TT, wc_tm[:, 0:D], scr["sp"][N:W], col_sp[:, N:W], "st")
        rstat_slot(x_sp_tp, NT, wr_sp[:, D : 2 * D], bb[:, 1:2],
                   r_sp[:, NT : 2 * NT], "st")
        cstat(x_sp_pt, NT, wc_sp[:, D : 2 * D], scr["tm"][0:N], col_tm[:, 0:N], "ts")
        rstat_slot(x_tm_tp, TT, wr_tm[:, 0:D], bb[:, 2:3], r_tm[:, 0:TT], "ts")
        cstat(x_tm_pt, TT, wc_tm[:, D : 2 * D], scr["tm"][N:W], col_tm[:, N:W], "tt")
        rstat_slot(x_tm_tp, TT, wr_tm[:, D : 2 * D], bb[:, 3:4],
                   r_tm[:, TT : 2 * TT], "tt")

        # ---- main loop: 20 output row-tiles of [128, 2560] ----------------
        def row_block(k, row0, col, st, nt, t, last=False):
            ot = outp.tile([128, W], BF16, name=f"ot{k}", tag="ot")
            nc.scalar.activation(
                ot[:, 0:N], col[:, 0:N], AF.Tanh, bias=st[:, t : t + 1]
            )
            nc.scalar.activation(
                ot[:, N:W], col[:, N:W], AF.Tanh, bias=st[:, nt + t : nt + t + 1]
            )
            nc.vector.tensor_scalar_max(ot[:], ot[:], 0.0)
            if last:
                # split the final store across all rings to shrink the tail
                for eng, lo, hi in ((nc.sync, 0, 1024), (nc.gpsimd, 1024, 2048),
                                    (nc.scalar, 2048, W)):
                    eng.dma_start(adj[row0 : row0 + 128, lo:hi], ot[:, lo:hi])
            else:
                eng = nc.sync if k % 2 == 0 else nc.gpsimd
                eng.dma_start(adj[row0 : row0 + 128, :], ot[:])

        for t in range(NT):
            row_block(t, 128 * t, col_sp, r_sp, NT, t)
        for t in range(TT):
            row_block(NT + t, N + 128 * t, col_tm, r_tm, TT, t,
                      last=(t == TT - 1))


def build_nc(num_devices=8):
    nc = bacc.Bacc(
        "TRN2",
        target_bir_lowering=False,
        debug=False,
        enable_asserts=True,
        num_devices=num_devices,
    )
    sp = nc.dram_tensor("spatial_nodes", (N, D), F32, kind="ExternalInput").ap()
    tm = nc.dram_tensor("temporal_nodes", (T, D), F32, kind="ExternalInput").ap()
    ws = {}
    for nm in QUADS:
        ws[f"w_{nm}"] = nc.dram_tensor(f"w_{nm}", (2 * D,), F32, kind="ExternalInput").ap()
        ws[f"b_{nm}"] = nc.dram_tensor(f"b_{nm}", (1,), F32, kind="ExternalInput").ap()
    scr = {
        "sp": nc.dram_tensor("scr_sp", (W,), BF16, kind="Internal").ap(),
        "tm": nc.dram_tensor("scr_tm", (W,), BF16, kind="Internal").ap(),
    }
    adj = nc.dram_tensor("adj", (W, W), BF16, kind="ExternalOutput").ap()

    with tile.TileContext(nc) as tc:
        _emit(tc, sp, tm, ws, scr, adj)
    nc.compile()
    return nc


def make_in_maps(inputs):
    in_maps = []
    for b in range(B):
        m = {
            "spatial_nodes": np.ascontiguousarray(inputs["spatial_nodes"][b], np.float32),
            "temporal_nodes": np.ascontiguousarray(inputs["temporal_nodes"][b], np.float32),
        }
        for nm in QUADS:
            m[f"w_{nm}"] = np.ascontiguousarray(inputs[f"w_{nm}"], np.float32)
            m[f"b_{nm}"] = np.ascontiguousarray(inputs[f"b_{nm}"], np.float32)
        in_maps.append(m)
    return in_maps


_NC = {}


def run(inputs, trace=False, trace_cores=None):
    if 8 not in _NC:
        _NC[8] = build_nc(8)
    res = run_bass_kernel_spmd(
        _NC[8], make_in_maps(inputs), core_ids=list(range(B)), trace=trace,
        trace_cores=trace_cores,
    )
    out = np.stack(
        [np.asarray(res.results[i]["adj"]).astype(np.float32) for i in range(B)],
        axis=0,
    )
    return out, res


def kernel(**inputs) -> np.ndarray:
    out, _ = run(inputs, trace=False)
    return out
